# revision 1
# baseline (speedup 1.0000x reference)
"""JointRetention Trainium2 kernel.

out[b] = ((xpos(X_b Wq) xpos_down(X_b Wk)^T) * D[b%17]) @ (X_b Wv)

Strategy:
  - Data-parallel over B*J=1088 across 8 cores (136 each; 136%17==0 so the
    joint index pattern is identical on every core).
  - rotate_every_two folded into host-precomputed W@R so xpos becomes
    Qx = (X Wq) * C + (X Wq R) * S  -- two matmuls + elementwise.
  - All tensors kept transposed on-chip (head dim on partitions) so
    S^T = Kx^T-major matmul feeds the masked A^T @ V matmul directly.
  - float32r matmuls (1 cycle/row when N>=256), batch pairs packed into
    512-wide tiles so every matmul free dim is 256/512.
"""

import numpy as np

L = 243
H = 256
J = 17
NCORES = 8
NB = 1088
BPC = NB // NCORES          # 136 batch rows per core
NPAIR = BPC // 2            # 68 pairs per core
SCALE_BASE = 512
CHUNK = 81

_cache = {}


def _host_tables(W_Q, W_K, W_V, gamma):
    f32 = np.float32
    # rot(y) = y @ R
    R = np.zeros((H, H), f32)
    idx = np.arange(0, H, 2)
    R[idx + 1, idx] = -1.0
    R[idx, idx + 1] = 1.0

    WQ = W_Q.astype(f32)
    WK = W_K.astype(f32)
    WV = W_V.astype(f32)
    WQR = (WQ @ R).astype(f32)
    WKR = (WK @ R).astype(f32)

    # xpos coefficient tables (L, H) then transposed to (H, L)
    half = H // 2
    base_scale = ((np.arange(0, H, 2, dtype=f32) + 0.4 * H) / (1.4 * H)).astype(f32)
    pos = np.arange(L, dtype=f32)
    scale = base_scale[None, :] ** (pos / SCALE_BASE)[:, None]        # (L, half)
    inv_freq = (1.0 / 10000.0 ** (np.arange(half, dtype=f32) / half)).astype(f32)
    sinus = pos[:, None] * inv_freq[None, :]
    sin, cos = np.sin(sinus).astype(f32), np.cos(sinus).astype(f32)

    def dup(m):
        return np.repeat(m, 2, axis=-1)

    CQ = dup(cos * scale).T.astype(f32)      # (H, L)
    SQ = dup(sin * scale).T.astype(f32)
    inv = (1.0 / scale).astype(f32)
    CK = dup(cos * inv).T.astype(f32)
    SK = dup(sin * inv).T.astype(f32)

    # pack weights: per h-chunk rows, cols [WQ | WQR | WK | WKR | WV] (1280)
    Wcat = np.concatenate([WQ, WQR, WK, WKR, WV], axis=1)            # (256, 1280)
    W_all = np.stack([Wcat[0:128], Wcat[128:256]], axis=0)           # (2, 128, 1280)

    # pack tables: per d-chunk rows, cols [CQ | SQ | CK | SK] each 512 wide
    # (pair layout: cols 0:243 = b0, 256:499 = b1, pads zero)
    def pack(tbl, dc):
        out = np.zeros((128, 512), f32)
        rows = tbl[dc * 128:(dc + 1) * 128]
        out[:, 0:L] = rows
        out[:, 256:256 + L] = rows
        return out

    CS = np.zeros((2, 128, 2048), f32)
    for dc in range(2):
        CS[dc, :, 0:512] = pack(CQ, dc)
        CS[dc, :, 512:1024] = pack(SQ, dc)
        CS[dc, :, 1024:1536] = pack(CK, dc)
        CS[dc, :, 1536:2048] = pack(SK, dc)

    # decay mask, transposed per joint, free dim padded to 256
    g = gamma.astype(f32)
    i = np.arange(L)[:, None]
    jj = np.arange(L)[None, :]
    allowed = jj < (i // CHUNK + 1) * CHUNK
    absd = np.abs(i - jj).astype(f32)
    D = g[:, None, None] ** absd[None]                               # (J, L, L)
    D = np.where(allowed[None], D, 0.0)
    D = np.where(np.isnan(D), 0.0, D).astype(f32)
    DT = np.zeros((J, L, 256), f32)
    DT[:, :, 0:L] = np.transpose(D, (0, 2, 1))                       # DT[j, m, l]

    ident = np.eye(128, dtype=f32)
    return W_all, CS, DT, ident


def _build():
    import concourse.bacc as bacc
    import concourse.mybir as mybir
    from concourse import tile

    dt = mybir.dt
    f32 = dt.float32
    f32r = dt.float32r

    nc = bacc.Bacc("TRN2", target_bir_lowering=False, debug=False,
                   num_devices=NCORES)
    X_d = nc.dram_tensor("X", (BPC, L, H), f32, kind="ExternalInput").ap()
    W_d = nc.dram_tensor("WALL", (2, 128, 1280), f32, kind="ExternalInput").ap()
    CS_d = nc.dram_tensor("CS", (2, 128, 2048), f32, kind="ExternalInput").ap()
    DT_d = nc.dram_tensor("DTAB", (J, L, 256), f32, kind="ExternalInput").ap()
    ID_d = nc.dram_tensor("IDEN", (128, 128), f32, kind="ExternalInput").ap()
    O_d = nc.dram_tensor("OUT", (BPC, L, H), f32, kind="ExternalOutput").ap()

    def rr(ap):
        return ap.bitcast(f32r)

    LSZ = (128, L - 128)          # l/m chunk sizes (128, 115)

    with tile.TileContext(nc) as tc:
        with (
            tc.tile_pool(name="const", bufs=1) as const,
            tc.tile_pool(name="xin", bufs=3) as xin,
            tc.tile_pool(name="work", bufs=2) as work,
            tc.tile_pool(name="pxt", bufs=2, space="PSUM") as pxt,
            tc.tile_pool(name="pproj", bufs=3, space="PSUM") as pproj,
            tc.tile_pool(name="pv", bufs=1, space="PSUM") as pv,
            tc.tile_pool(name="pso", bufs=2, space="PSUM") as pso,
        ):
            # ---- constants ----
            w_sb = [const.tile([128, 1280], f32, name=f"w{h}", tag=f"w{h}") for h in range(2)]
            cs_sb = [const.tile([128, 2048], f32, name=f"cs{d}", tag=f"cs{d}") for d in range(2)]
            ident = const.tile([128, 128], f32, name="ident", tag="ident")
            dt_sb = [[const.tile([LSZ[mc], 256], f32, name=f"dt{j}_{mc}", tag=f"dt{j}_{mc}")
                      for mc in range(2)] for j in range(J)]
            w_r = [const.tile([128, 1280], f32r, name=f"wr{h}", tag=f"wr{h}")
                   for h in range(2)]
            for h in range(2):
                nc.sync.dma_start(w_sb[h][:], W_d[h])
                nc.sync.dma_start(cs_sb[h][:], CS_d[h])
                nc.scalar.copy(w_r[h][:], w_sb[h][:])
            nc.sync.dma_start(ident[:], ID_d[:])
            for j in range(J):
                for mc in range(2):
                    nc.sync.dma_start(dt_sb[j][mc][:],
                                      DT_d[j, mc * 128:mc * 128 + LSZ[mc], :])

            for t in range(NPAIR):
                b0 = 2 * t
                joints = (b0 % J, (b0 + 1) % J)

                # ---- load X pair ----
                xt_in = []
                for k in range(2):
                    row = []
                    for lc in range(2):
                        tl = xin.tile([LSZ[lc], H], f32, name=f"x{k}{lc}", tag=f"x{k}{lc}")
                        nc.sync.dma_start(
                            tl[:], X_d[b0 + k, lc * 128:lc * 128 + LSZ[lc], :])
                        row.append(tl)
                    xt_in.append(row)

                # ---- transpose X -> XT (h on partitions), pair packed ----
                # psum cols: b0 at 0:243, b1 at 243:486
                xt_sb = []
                for h in range(2):
                    ps = pxt.tile([128, 512], f32, name="xtp", tag="xtp")
                    for k in range(2):
                        for lc in range(2):
                            col = k * L + lc * 128
                            nc.tensor.transpose(
                                ps[:, col:col + LSZ[lc]],
                                xt_in[k][lc][:, h * 128:(h + 1) * 128],
                                ident[0:LSZ[lc], 0:LSZ[lc]],
                            )
                    sb = work.tile([128, 512], f32r, name=f"xt{h}", tag=f"xt{h}")
                    # repack: b0 -> 0:243, b1 -> 256:499 (pads never read as
                    # real data; CS tables carry zeros in pad cols)
                    nc.scalar.copy(sb[:, 0:L], ps[:, 0:L])
                    nc.scalar.copy(sb[:, 256:256 + L], ps[:, L:2 * L])
                    xt_sb.append(sb)

                # ---- V = X @ Wv  (natural layout: l on partitions) ----
                v_sb = []
                for lc in range(2):
                    ps = pv.tile([128, 512], f32, name="vp", tag="vp")
                    for k in range(2):
                        for h in range(2):
                            nc.tensor.matmul(
                                ps[0:LSZ[lc], k * 256:k * 256 + 256],
                                xt_sb[h][:, k * 256 + lc * 128:
                                        k * 256 + lc * 128 + LSZ[lc]],
                                w_r[h][:, 1024:1280],
                                start=(h == 0), stop=(h == 1),
                            )
                    sb = work.tile([128, 512], f32r, name=f"v{lc}", tag=f"v{lc}")
                    nc.scalar.copy(sb[0:LSZ[lc], :], ps[0:LSZ[lc], :])
                    v_sb.append(sb)

                # ---- projections (transposed: d on partitions) + xpos ----
                # tensors: 0=Q, 1=QR, 2=K, 3=KR ; combine pairs (0,1)->Qx, (2,3)->Kx
                qx, kx = [], []
                for pair_i, dst in ((0, qx), (2, kx)):
                    for dc in range(2):
                        ps_a = pproj.tile([128, 512], f32, name="proj", tag="proj")
                        ps_b = pproj.tile([128, 512], f32, name="proj", tag="proj")
                        for h in range(2):
                            nc.tensor.matmul(
                                ps_a[:],
                                w_r[h][:, pair_i * 256 + dc * 128:
                                       pair_i * 256 + dc * 128 + 128],
                                xt_sb[h][:],
                                start=(h == 0), stop=(h == 1),
                            )
                        for h in range(2):
                            nc.tensor.matmul(
                                ps_b[:],
                                w_r[h][:, (pair_i + 1) * 256 + dc * 128:
                                       (pair_i + 1) * 256 + dc * 128 + 128],
                                xt_sb[h][:],
                                start=(h == 0), stop=(h == 1),
                            )
                        # xpos: out = ps_a * C + ps_b * S
                        cbase = (0 if pair_i == 0 else 1024)
                        t1 = work.tile([128, 512], f32, name="t1", tag="t1")
                        t2 = work.tile([128, 512], f32, name="t2", tag="t2")
                        nc.vector.tensor_mul(
                            t1[:], ps_a[:], cs_sb[dc][:, cbase:cbase + 512])
                        nc.vector.tensor_mul(
                            t2[:], ps_b[:], cs_sb[dc][:, cbase + 512:cbase + 1024])
                        out = work.tile([128, 512], f32r,
                                        name=f"{'qx' if pair_i == 0 else 'kx'}{dc}",
                                        tag=f"{'qx' if pair_i == 0 else 'kx'}{dc}")
                        nc.gpsimd.tensor_add(out[:], t1[:], t2[:])
                        dst.append(out)

                # ---- attention per batch element ----
                for k in range(2):
                    jt = joints[k]
                    at = []
                    for mc in range(2):
                        msz = LSZ[mc]
                        ps = pso.tile([128, 256], f32, name="so", tag="so")
                        for dc in range(2):
                            nc.tensor.matmul(
                                ps[0:msz, :],
                                kx[dc][:, k * 256 + mc * 128:
                                       k * 256 + mc * 128 + msz],
                                qx[dc][:, k * 256:k * 256 + 256],
                                start=(dc == 0), stop=(dc == 1),
                            )
                        a = work.tile([LSZ[mc], 256], f32r, name=f"at{mc}", tag=f"at{mc}")
                        nc.vector.tensor_mul(a[:], ps[0:msz, :], dt_sb[jt][mc][:])
                        at.append(a)
                    for lc in range(2):
                        lsz = LSZ[lc]
                        ps = pso.tile([128, 256], f32, name="so", tag="so")
                        for mc in range(2):
                            nc.tensor.matmul(
                                ps[0:lsz, :],
                                at[mc][:, lc * 128:lc * 128 + lsz],
                                v_sb[mc][0:LSZ[mc], k * 256:k * 256 + 256],
                                start=(mc == 0), stop=(mc == 1),
                            )
                        ob = work.tile([128, 256], f32, name=f"ob{lc}",
                                       tag=f"ob{lc}")
                        nc.scalar.copy(ob[0:lsz, :], ps[0:lsz, :])
                        nc.sync.dma_start(
                            O_d[b0 + k, lc * 128:lc * 128 + lsz, :],
                            ob[0:lsz, :])

    nc.compile()
    return nc


def _get_nc():
    if "nc" not in _cache:
        _cache["nc"] = _build()
    return _cache["nc"]


def _run(in_maps, trace=False):
    from concourse import bass_utils
    nc = _get_nc()
    return bass_utils.run_bass_kernel_spmd(
        nc, in_maps, core_ids=list(range(NCORES)), trace=trace)


def kernel(X, W_Q, W_K, W_V, gamma, _trace=False):
    X = np.asarray(X, np.float32)
    W_all, CS, DT, ident = _host_tables(
        np.asarray(W_Q, np.float32), np.asarray(W_K, np.float32),
        np.asarray(W_V, np.float32), np.asarray(gamma, np.float32))

    in_maps = []
    for c in range(NCORES):
        in_maps.append({
            "X": np.ascontiguousarray(X[c * BPC:(c + 1) * BPC]),
            "WALL": W_all, "CS": CS, "DTAB": DT, "IDEN": ident,
        })
    res = _run(in_maps, trace=_trace)
    out = np.concatenate([r["OUT"] for r in res.results], axis=0)
    if _trace:
        _cache["last_result"] = res
    return out



# revision 2
# speedup vs baseline: 1.1663x; 1.1663x over previous
"""JointRetention Trainium2 kernel (v2).

out[b] = ((xpos(X_b Wq) xpos_down(X_b Wk)^T) * D[b%17]) @ (X_b Wv)

Strategy (v2):
  - Data-parallel over B*J=1088 across 8 cores (136 each; 136%17==0 so the
    joint pattern is identical on every core). Pairs of batches packed into
    486-wide tiles.
  - X is pre-transposed AND pre-cast to bf16 on the host: the kernel loads
    XT[h, l] tiles directly (no on-chip transposes, half the load bytes).
  - All matmuls bf16 (1 cycle/row on the PE at any free size, vs fp32r
    needing >=256): proj -> Yq,Yk (transposed), V (natural).
  - xpos via the identity rot(Y*S) = rot(Y)*S (duplicate_interleave makes
    sin/cos pairwise equal), so:  Qx = Y*C + PE_rot(Y*S)  where PE_rot is a
    128x128 +-1 permutation matmul. No host-side W@R double projections.
  - mask multiply fused with the PSUM->SBUF move of the scores.
  - Output stored as bf16 (rel-err budget 2e-2, measured ~6.7e-3 end to end
    in a full-pipeline numpy simulation) and up-cast on the host.
  - Elementwise spread over DVE (PSUM-reading ops: combines + mask), GPSIMD
    (pure-SBUF bf16 muls), ACT (PSUM->SBUF copies). Loads issued on the sync
    HWDGE ring, stores on the scalar ring to spread DMA descriptor load.
"""

import numpy as np
import ml_dtypes

L = 243
H = 256
J = 17
NCORES = 8
NB = 1088
BPC = NB // NCORES          # 136 batch rows per core
NPAIR = BPC // 2            # 68 pairs per core
SCALE_BASE = 512
CHUNK = 81
L2 = 2 * L                  # 486: pair-packed free dim
LSZ = (128, L - 128)        # 128/115 chunks of L

_bf16 = ml_dtypes.bfloat16
_cache = {}


def _host_tables(W_Q, W_K, W_V, gamma):
    f32 = np.float32

    # packed weights [hc][128 h, 768]: cols [Wq | Wk | Wv]
    Wcat = np.concatenate([W_Q, W_K, W_V], axis=1).astype(f32)     # (256, 768)
    WC = np.stack([Wcat[0:128], Wcat[128:256]], axis=0).astype(_bf16)

    # rot permutation matmul: out[m] = sum_k PM[k, m] u[k]
    # out[2i] = -u[2i+1], out[2i+1] = u[2i]
    PM = np.zeros((128, 128), f32)
    idx = np.arange(0, 128, 2)
    PM[idx + 1, idx] = -1.0
    PM[idx, idx + 1] = 1.0
    PM = PM.astype(_bf16)

    # xpos tables (transposed: [d, l]), pair-packed to 486 cols
    half = H // 2
    base_scale = ((np.arange(0, H, 2, dtype=f32) + 0.4 * H) / (1.4 * H)).astype(f32)
    pos = np.arange(L, dtype=f32)
    scale = base_scale[None, :] ** (pos / SCALE_BASE)[:, None]        # (L, half)
    inv_freq = (1.0 / 10000.0 ** (np.arange(half, dtype=f32) / half)).astype(f32)
    sinus = pos[:, None] * inv_freq[None, :]
    sin, cos = np.sin(sinus).astype(f32), np.cos(sinus).astype(f32)

    def dup(m):
        return np.repeat(m, 2, axis=-1)

    tables = [dup(cos * scale), dup(sin * scale),          # q: C, S
              dup(cos / scale), dup(sin / scale)]          # k: C, S
    CS = np.zeros((2, 2, 2, 128, L2), _bf16)               # [tensor, coef, dc]
    for ti in range(2):
        for coef in range(2):
            T = tables[ti * 2 + coef].T.astype(f32)        # (256, L)
            for dc in range(2):
                CS[ti, coef, dc] = np.tile(
                    T[dc * 128:(dc + 1) * 128], (1, 2)).astype(_bf16)

    # decay mask, transposed per joint: DT[j][m, l] = D[j][l, m]
    g = gamma.astype(f32)
    i = np.arange(L)[:, None]
    jj = np.arange(L)[None, :]
    allowed = jj < (i // CHUNK + 1) * CHUNK
    absd = np.abs(i - jj).astype(f32)
    D = g[:, None, None] ** absd[None]
    D = np.where(allowed[None], D, 0.0)
    D = np.where(np.isnan(D), 0.0, D).astype(f32)
    DT = np.zeros((J, 2, 128, L), f32)
    for j in range(J):
        Dt = D[j].T                                        # [m, l]
        for mc in range(2):
            DT[j, mc, 0:LSZ[mc]] = Dt[mc * 128:mc * 128 + LSZ[mc]]
    DT = DT.astype(_bf16)

    return WC, PM, CS, DT


def _pack_x(Xc):
    # Xc: (136, 243, 256) f32 -> (68, 2(hc), 128, 486) bf16, cols b0|b1
    Xt = Xc.transpose(0, 2, 1)                 # (136, 256, 243)
    Xt = Xt.reshape(NPAIR, 2, 2, 128, L)       # (t, kb, hc, p, l)
    Xt = Xt.transpose(0, 2, 3, 1, 4)           # (t, hc, p, kb, l)
    return np.ascontiguousarray(Xt.reshape(NPAIR, 2, 128, L2)).astype(_bf16)


def _unpack_out(buf):
    # buf: (68, 2, 128, 512) bf16 -> (136, 243, 256) f32
    b = buf.astype(np.float32)
    p1 = b[:, :, :, 0:256]                     # l 0:128
    p2 = b[:, :, 0:LSZ[1], 256:512]            # l 128:243
    out = np.concatenate([p1, p2], axis=2)     # (68, 2, 243, 256)
    return out.reshape(BPC, L, H)


def _build():
    import concourse.bacc as bacc
    import concourse.mybir as mybir
    from concourse import tile

    dt = mybir.dt
    f32 = dt.float32
    bf16 = dt.bfloat16

    nc = bacc.Bacc("TRN2", target_bir_lowering=False, debug=False,
                   num_devices=NCORES)
    XT_d = nc.dram_tensor("XT", (NPAIR, 2, 128, L2), bf16, kind="ExternalInput").ap()
    WC_d = nc.dram_tensor("WC", (2, 128, 768), bf16, kind="ExternalInput").ap()
    PM_d = nc.dram_tensor("PM", (128, 128), bf16, kind="ExternalInput").ap()
    CS_d = nc.dram_tensor("CS", (2, 2, 2, 128, L2), bf16, kind="ExternalInput").ap()
    DT_d = nc.dram_tensor("DTAB", (J, 2, 128, L), bf16, kind="ExternalInput").ap()
    O_d = nc.dram_tensor("OUT", (NPAIR, 2, 128, 512), bf16, kind="ExternalOutput").ap()

    with tile.TileContext(nc) as tc:
        with (
            tc.tile_pool(name="const", bufs=1) as const,
            tc.tile_pool(name="xin", bufs=3) as xin,
            tc.tile_pool(name="ysb", bufs=2) as ysb,
            tc.tile_pool(name="uv", bufs=2) as uv,
            tc.tile_pool(name="qk", bufs=2) as qk,
            tc.tile_pool(name="vsb", bufs=2) as vsb,
            tc.tile_pool(name="atp", bufs=2) as atp,
            tc.tile_pool(name="osb", bufs=2) as osb,
            tc.tile_pool(name="py", bufs=1, space="PSUM") as py,
            tc.tile_pool(name="pr", bufs=1, space="PSUM") as pr,
            tc.tile_pool(name="pv", bufs=1, space="PSUM") as pv,
            tc.tile_pool(name="ps", bufs=1, space="PSUM") as ps,
            tc.tile_pool(name="po", bufs=1, space="PSUM") as po,
        ):
            # ---- constants ----
            wc = [const.tile([128, 768], bf16, name=f"wc{h}", tag=f"wc{h}")
                  for h in range(2)]
            pm = const.tile([128, 128], bf16, name="pm", tag="pm")
            cs = {}
            for ti in range(2):
                for coef in range(2):
                    for dc in range(2):
                        t_ = const.tile([128, L2], bf16,
                                        name=f"cs{ti}{coef}{dc}",
                                        tag=f"cs{ti}{coef}{dc}")
                        nc.sync.dma_start(t_[:], CS_d[ti, coef, dc])
                        cs[(ti, coef, dc)] = t_
            dts = [[const.tile([128, L], bf16, name=f"dt{j}_{mc}",
                               tag=f"dt{j}_{mc}") for mc in range(2)]
                   for j in range(J)]
            for h in range(2):
                nc.sync.dma_start(wc[h][:], WC_d[h])
            nc.sync.dma_start(pm[:], PM_d[:])
            for j in range(J):
                for mc in range(2):
                    nc.sync.dma_start(dts[j][mc][:], DT_d[j, mc])

            for t in range(NPAIR):
                joints = ((2 * t) % J, (2 * t + 1) % J)

                # ---- load XT pair ----
                xt = []
                for hc in range(2):
                    tl = xin.tile([128, L2], bf16, name=f"x{hc}", tag=f"x{hc}")
                    nc.sync.dma_start(tl[:], XT_d[t, hc])
                    xt.append(tl)

                # ---- Yq, Yk projections (transposed: d on partitions) ----
                y = {}
                for ti in range(2):
                    toff = ti * 256
                    for dc in range(2):
                        pyt = py.tile([128, 512], f32, name="pyt", tag=f"py{dc}")
                        for hc in range(2):
                            nc.tensor.matmul(
                                pyt[:, 0:L2],
                                wc[hc][:, toff + dc * 128: toff + dc * 128 + 128],
                                xt[hc][:],
                                start=(hc == 0), stop=(hc == 1),
                            )
                        yt = ysb.tile([128, L2], bf16, name=f"y{ti}{dc}",
                                      tag=f"y{ti}{dc}")
                        nc.scalar.copy(yt[:], pyt[:, 0:L2])
                        y[(ti, dc)] = yt

                # ---- xpos tables: u = Y*S (rot operand), v = Y*C ----
                u, v = {}, {}
                nmul = 0
                for ti in range(2):
                    for dc in range(2):
                        ut = uv.tile([128, L2], bf16, name=f"u{ti}{dc}",
                                     tag=f"u{ti}{dc}")
                        vt = uv.tile([128, L2], bf16, name=f"v{ti}{dc}",
                                     tag=f"v{ti}{dc}")
                        # 3 of 8 muls on gpsimd to offload the DVE
                        eng_u = nc.gpsimd if nmul in (0, 2, 4) else nc.vector
                        eng_u.tensor_mul(ut[:], y[(ti, dc)][:],
                                         cs[(ti, 1, dc)][:])
                        nc.vector.tensor_mul(vt[:], y[(ti, dc)][:],
                                             cs[(ti, 0, dc)][:])
                        nmul += 2
                        u[(ti, dc)] = ut
                        v[(ti, dc)] = vt

                # ---- rot matmul + combine: Qx = v + rot(u) ----
                qx, kx = [], []
                for ti, dst in ((0, qx), (1, kx)):
                    for dc in range(2):
                        prt = pr.tile([128, 512], f32, name="prt", tag=f"pr{dc}")
                        nc.tensor.matmul(prt[:, 0:L2], pm[:], u[(ti, dc)][:],
                                         start=True, stop=True)
                        qt = qk.tile([128, L2], bf16,
                                     name=f"{'qx' if ti == 0 else 'kx'}{dc}",
                                     tag=f"{'qx' if ti == 0 else 'kx'}{dc}")
                        nc.vector.tensor_add(qt[:], v[(ti, dc)][:],
                                             prt[:, 0:L2])
                        dst.append(qt)

                # ---- V projection (natural: l on partitions) ----
                vs = []
                for kb in range(2):
                    pvt = pv.tile([128, 512], f32, name="pvt", tag=f"pv{kb}")
                    for lc in range(2):
                        lsz = LSZ[lc]
                        for hc in range(2):
                            nc.tensor.matmul(
                                pvt[0:lsz, lc * 256: lc * 256 + 256],
                                xt[hc][:, kb * L + lc * 128: kb * L + lc * 128 + lsz],
                                wc[hc][:, 512:768],
                                start=(hc == 0), stop=(hc == 1),
                            )
                    vt = vsb.tile([128, 512], bf16, name=f"v{kb}", tag=f"v{kb}")
                    nc.scalar.copy(vt[:], pvt[:])
                    vs.append(vt)

                # ---- scores^T, mask, AV, store per batch ----
                for kb in range(2):
                    jk = joints[kb]
                    pst = ps.tile([128, 512], f32, name="pst", tag="ps")
                    for mc in range(2):
                        msz = LSZ[mc]
                        for dc in range(2):
                            nc.tensor.matmul(
                                pst[0:msz, mc * L: mc * L + L],
                                kx[dc][:, kb * L + mc * 128: kb * L + mc * 128 + msz],
                                qx[dc][:, kb * L: kb * L + L],
                                start=(dc == 0), stop=(dc == 1),
                            )
                    ats = []
                    for mc in range(2):
                        msz = LSZ[mc]
                        att = atp.tile([128, L], bf16, name=f"at{mc}",
                                       tag=f"at{mc}")
                        nc.vector.tensor_mul(att[0:msz, :],
                                             pst[0:msz, mc * L: mc * L + L],
                                             dts[jk][mc][0:msz, :])
                        ats.append(att)
                    pot = po.tile([128, 512], f32, name="pot", tag="po")
                    for lc in range(2):
                        lsz = LSZ[lc]
                        for mc in range(2):
                            msz = LSZ[mc]
                            nc.tensor.matmul(
                                pot[0:lsz, lc * 256: lc * 256 + 256],
                                ats[mc][0:msz, lc * 128: lc * 128 + lsz],
                                vs[kb][0:msz, mc * 256: mc * 256 + 256],
                                start=(mc == 0), stop=(mc == 1),
                            )
                    ot = osb.tile([128, 512], bf16, name=f"o{kb}", tag=f"o{kb}")
                    nc.scalar.copy(ot[:], pot[:])
                    nc.scalar.dma_start(O_d[t, kb], ot[:])

    nc.compile()
    return nc


def _get_nc():
    if "nc" not in _cache:
        _cache["nc"] = _build()
    return _cache["nc"]


def _run(in_maps, trace=False):
    from concourse import bass_utils
    nc = _get_nc()
    return bass_utils.run_bass_kernel_spmd(
        nc, in_maps, core_ids=list(range(NCORES)), trace=trace)


def kernel(X, W_Q, W_K, W_V, gamma, _trace=False):
    X = np.asarray(X, np.float32)
    WC, PM, CS, DT = _host_tables(
        np.asarray(W_Q, np.float32), np.asarray(W_K, np.float32),
        np.asarray(W_V, np.float32), np.asarray(gamma, np.float32))

    in_maps = []
    for c in range(NCORES):
        in_maps.append({
            "XT": _pack_x(X[c * BPC:(c + 1) * BPC]),
            "WC": WC, "PM": PM, "CS": CS, "DTAB": DT,
        })
    res = _run(in_maps, trace=_trace)
    out = np.concatenate([_unpack_out(r["OUT"]) for r in res.results], axis=0)
    if _trace:
        _cache["last_result"] = res
    return out
